# revision 22
# baseline (speedup 1.0000x reference)
"""Trainium2 Bass kernel for graph-transformer message passing (TransformerConv).

Strategy (8 NeuronCores, SPMD, no collectives):
  - Host shards edges across cores by contiguous dst-node ranges (49 blocks
    of 128 nodes per core) and, within each block, splits edges by src half
    (src < 25088 vs >= 25088).
  - The K||V node table lives IN SBUF, interleaved [128 feat, 25088 nodes, 2]
    bf16 (entry d=0 is K[n,f], d=1 is V[n,f]) - one src half at a time.
    Per-edge K and V rows are fetched with a single gpsimd ap_gather per
    (pass, block): one index per edge, no DMA descriptors. The idx slab
    slices start at 8-column (128-index) boundaries - the gather ucode
    misreads idx columns at unaligned offsets.
  - Two passes over the blocks (one per src half). Pass A_h builds the table
    half with dense matmuls from the x stream; pass B_h computes per-edge
    alpha (per-head PE matmuls kjT^T @ qT with head-masked q operands, a
    one-hot dst mask built on-device, and a mult+reduce), softmax numerators
    ex, and accumulates the weighted-V segment sums into an SBUF
    accumulator. The epilogue (normalize, beta skip, proj) runs per block at
    the end of pass 1.
  - DMA is slab-shaped (large contiguous descriptors): ~2.5k descriptors
    per core vs ~276k for a DRAM-gather formulation.
"""

import sys

sys.path.insert(0, "/opt/trn_rl_repo")

import numpy as np

N, E, D, H, ED = 50000, 600000, 128, 2, 5
C = D // H
NCORES = 8
P = 128
NB = 49                 # node blocks per core
L = NB * P              # 6272 local nodes per core
NPAD = 392 * P          # 50176 padded node count
HALF = NPAD // 2        # 25088 nodes per table pass
QSCALE = 0.125          # 1/sqrt(C)


def _bf16(a):
    import ml_dtypes

    return np.asarray(a, dtype=np.float32).astype(ml_dtypes.bfloat16)


def _wrap16(flat):
    # index i -> [i%16, i//16], replicated over the 8 partition groups
    w = flat.reshape(-1, 16).T.astype(np.int16)
    return np.tile(w, (8, 1))


def _prep_host(x, edge_index, edge_attr, Wq, bq, Wk, bk, Wv, bv, We,
               Wskip, bskip, Wbeta, Wproj, bproj):
    src = np.asarray(edge_index[0], dtype=np.int64)
    dst = np.asarray(edge_index[1], dtype=np.int64)
    ea = np.asarray(edge_attr, dtype=np.float32)

    core_of = dst // L
    blk_of = (dst % L) // P
    half_of = src // HALF

    order = np.lexsort((half_of, blk_of, core_of))
    s_src, s_dst = src[order], dst[order]
    s_core, s_blk, s_half = core_of[order], blk_of[order], half_of[order]
    s_ea = ea[order]

    # group boundaries per (core, blk, half)
    gid = (s_core * NB + s_blk) * 2 + s_half
    counts = np.bincount(gid, minlength=NCORES * NB * 2).reshape(NCORES, NB, 2)
    ni = (-(-counts // 16) * 16).astype(np.int64)         # 16-granular pad

    # per-core slab sizes (program is shared; pad to the max across cores)
    ni_max = np.maximum(ni.max(axis=0), 16)               # [NB, 2]
    T_max = -(-ni_max // P)
    colw = (-(-(ni_max // 16) // 8) * 8).astype(np.int64)  # aligned col width
    sumT = [int(T_max[:, h].sum()) for h in range(2)]
    sumCW = [int(colw[:, h].sum()) for h in range(2)]
    toff = np.zeros((NB, 2), np.int64)
    qoff = np.zeros((NB, 2), np.int64)
    for h in range(2):
        toff[:, h] = np.concatenate([[0], np.cumsum(T_max[:, h])[:-1]])
        qoff[:, h] = np.concatenate([[0], np.cumsum(colw[:, h])[:-1]])
    sumT_all = sumT[0] + sumT[1]

    idx = [np.zeros((NCORES, P, sumCW[h]), np.int16) for h in range(2)]
    dstl = np.full((NCORES, P, sumT_all), 300.0, np.float32)
    eaT6 = np.zeros((NCORES, 6, sumT_all * P), np.float32)
    eaT6[:, 5, :] = 1.0

    starts = np.concatenate([[0], np.cumsum(counts.reshape(-1))])
    for c in range(NCORES):
        for b in range(NB):
            for h in range(2):
                g = (c * NB + b) * 2 + h
                lo, hi = starts[g], starts[g + 1]
                cnt = hi - lo
                n = max(int(ni[c, b, h]), 16)
                gsrc = np.zeros(n, np.int64)
                gsrc[:cnt] = s_src[lo:hi] - h * HALF
                idx[h][c, :, qoff[b, h]:qoff[b, h] + n // 16] = _wrap16(gsrc)
                t0 = toff[b, h] + (sumT[0] if h else 0)
                dstl[c, (np.arange(cnt) % P),
                     t0 + np.arange(cnt) // P] = (s_dst[lo:hi] - c * L - b * P)
                eaT6[c, :5, t0 * P:t0 * P + cnt] = s_ea[lo:hi].T
    xpad = np.zeros((NPAD, D), dtype=np.float32)
    xpad[:N] = np.asarray(x, dtype=np.float32)
    xT = _bf16(xpad.T)                                   # [128, NPAD]

    xTloc = np.zeros((NCORES, D, L), dtype=np.float32)
    for c in range(NCORES):
        hi = min(N, (c + 1) * L)
        if hi > c * L:
            xTloc[c, :, : hi - c * L] = xpad[c * L: hi].T
    xTloc = _bf16(xTloc)

    Wb = np.asarray(Wbeta, dtype=np.float32).reshape(3, D)
    WeT = np.asarray(We, dtype=np.float32).T             # [5, 128]
    consts = {
        "wkt": _bf16(np.asarray(Wk).T), "wvt": _bf16(np.asarray(Wv).T),
        "wqt": _bf16(np.asarray(Wq).T),
        "wskt": _bf16(np.asarray(Wskip).T), "wprt": _bf16(np.asarray(Wproj).T),
        "we2k": _bf16(np.concatenate([WeT, np.asarray(bk, np.float32)
                                      .reshape(1, D)], 0)),
        "we2v": _bf16(np.concatenate([WeT, np.asarray(bv, np.float32)
                                      .reshape(1, D)], 0)),
        "bkcol": np.asarray(bk, np.float32).reshape(D, 1),
        "bvcol": np.asarray(bv, np.float32).reshape(D, 1),
        "qsc0": (QSCALE * (np.arange(D) < C)).astype(np.float32).reshape(D, 1),
        "qsc1": (QSCALE * (np.arange(D) >= C)).astype(np.float32).reshape(D, 1),
        "qbc0": (np.asarray(bq, np.float32) * QSCALE
                 * (np.arange(D) < C)).reshape(D, 1).astype(np.float32),
        "qbc1": (np.asarray(bq, np.float32) * QSCALE
                 * (np.arange(D) >= C)).reshape(D, 1).astype(np.float32),
        "bskrow": _bf16(np.asarray(bskip).reshape(1, D)),
        "bprrow": _bf16(np.asarray(bproj).reshape(1, D)),
        "wb1rep": _bf16(np.tile((Wb[0] + Wb[2]).reshape(1, D), (P, 1))),
        "wb2rep": _bf16(np.tile((Wb[1] - Wb[2]).reshape(1, D), (P, 1))),
        "iota": _bf16(np.tile(np.arange(P, dtype=np.float32).reshape(1, P),
                              (P, 1))),
        "onesrow": _bf16(np.ones((1, D), dtype=np.float32)),
    }

    per_core = []
    for c in range(NCORES):
        m = dict(consts)
        m["xt"] = xT
        m["xtloc"] = xTloc[c]
        m["idx0"] = idx[0][c]
        m["idx1"] = idx[1][c]
        m["dstl"] = _bf16(dstl[c])
        m["eat6"] = _bf16(eaT6[c])
        per_core.append(m)
    meta = dict(
        ni=[[int(ni_max[b, h]) for b in range(NB)] for h in range(2)],
        T=[[int(T_max[b, h]) for b in range(NB)] for h in range(2)],
        toff=[[int(toff[b, h]) for b in range(NB)] for h in range(2)],
        qoff=[[int(qoff[b, h]) for b in range(NB)] for h in range(2)],
        sumT=sumT, sumCW=sumCW,
    )
    return per_core, meta


def _build_program(meta):
    import os
    STAGE = int(os.environ.get("BISECT_STAGE", "9"))
    DUMP = os.environ.get("DUMP_TENSOR", "")
    DH = int(os.environ.get("DUMP_H", "0"))
    DB = int(os.environ.get("DUMP_B", "0"))
    DT = int(os.environ.get("DUMP_T", "0"))
    import concourse.bacc as bacc
    import concourse.mybir as mybir
    import concourse.tile as tile
    from concourse.masks import make_identity

    fp32 = mybir.dt.float32
    bf16 = mybir.dt.bfloat16
    i16 = mybir.dt.int16
    AX = mybir.AluOpType
    AF = mybir.ActivationFunctionType

    sumT = meta["sumT"]
    sumCW = meta["sumCW"]
    sumT_all = sumT[0] + sumT[1]

    nc = bacc.Bacc("TRN2", target_bir_lowering=False, num_devices=NCORES)

    xt = nc.declare_dram_parameter("xt", [D, NPAD], bf16, isOutput=False)
    xtloc = nc.declare_dram_parameter("xtloc", [D, L], bf16, isOutput=False)
    idx0 = nc.declare_dram_parameter("idx0", [P, sumCW[0]], i16, isOutput=False)
    idx1 = nc.declare_dram_parameter("idx1", [P, sumCW[1]], i16, isOutput=False)
    dstl = nc.declare_dram_parameter("dstl", [P, sumT_all], bf16, isOutput=False)
    eat6 = nc.declare_dram_parameter("eat6", [6, sumT_all * P], bf16, isOutput=False)
    wkt = nc.declare_dram_parameter("wkt", [D, D], bf16, isOutput=False)
    wvt = nc.declare_dram_parameter("wvt", [D, D], bf16, isOutput=False)
    wqt = nc.declare_dram_parameter("wqt", [D, D], bf16, isOutput=False)
    wskt = nc.declare_dram_parameter("wskt", [D, D], bf16, isOutput=False)
    wprt = nc.declare_dram_parameter("wprt", [D, D], bf16, isOutput=False)
    we2k = nc.declare_dram_parameter("we2k", [6, D], bf16, isOutput=False)
    we2v = nc.declare_dram_parameter("we2v", [6, D], bf16, isOutput=False)
    bkcol = nc.declare_dram_parameter("bkcol", [D, 1], fp32, isOutput=False)
    bvcol = nc.declare_dram_parameter("bvcol", [D, 1], fp32, isOutput=False)
    qsc0 = nc.declare_dram_parameter("qsc0", [D, 1], fp32, isOutput=False)
    qsc1 = nc.declare_dram_parameter("qsc1", [D, 1], fp32, isOutput=False)
    qbc0 = nc.declare_dram_parameter("qbc0", [D, 1], fp32, isOutput=False)
    qbc1 = nc.declare_dram_parameter("qbc1", [D, 1], fp32, isOutput=False)
    bskrow = nc.declare_dram_parameter("bskrow", [1, D], bf16, isOutput=False)
    bprrow = nc.declare_dram_parameter("bprrow", [1, D], bf16, isOutput=False)
    wb1rep = nc.declare_dram_parameter("wb1rep", [P, D], bf16, isOutput=False)
    wb2rep = nc.declare_dram_parameter("wb2rep", [P, D], bf16, isOutput=False)
    iota = nc.declare_dram_parameter("iota", [P, P], bf16, isOutput=False)
    onesrow = nc.declare_dram_parameter("onesrow", [1, D], bf16, isOutput=False)
    out = nc.declare_dram_parameter("out", [L, D], fp32, isOutput=True)

    with tile.TileContext(nc) as tc:
        with tc.tile_pool(name="res", bufs=1) as res, \
             tc.tile_pool(name="pa", bufs=1) as pa, \
             tc.tile_pool(name="pap", bufs=1, space="PSUM") as pap, \
             tc.tile_pool(name="pb", bufs=2) as pb, \
             tc.tile_pool(name="pbs", bufs=2) as pbs, \
             tc.tile_pool(name="pbe", bufs=1, space="PSUM") as pbe, \
             tc.tile_pool(name="pba", bufs=1, space="PSUM") as pba, \
             tc.tile_pool(name="pbt", bufs=2, space="PSUM") as pbt, \
             tc.tile_pool(name="pbv", bufs=1, space="PSUM") as pbv, \
             tc.tile_pool(name="pbq", bufs=1, space="PSUM") as pbq:

            # ---- resident tensors ----
            tbl = res.tile([P, HALF * 2], bf16)
            xtloc_sb = res.tile([D, L], bf16)
            nc.sync.dma_start(out=xtloc_sb[:], in_=xtloc[:])
            idx0_sb = res.tile([P, sumCW[0]], i16)
            idx1_sb = res.tile([P, sumCW[1]], i16)
            idx_sb = [idx0_sb, idx1_sb]
            nc.sync.dma_start(out=idx_sb[0][:], in_=idx0[:])
            nc.sync.dma_start(out=idx_sb[1][:], in_=idx1[:])
            dstl_sb = res.tile([P, sumT_all], bf16)
            nc.sync.dma_start(out=dstl_sb[:], in_=dstl[:])
            acc_sb = res.tile([P, NB * 132], fp32)
            ex_sb = res.tile([P, 2 * sumT_all], bf16)

            wk_sb = res.tile([D, D], bf16)
            nc.sync.dma_start(out=wk_sb[:], in_=wkt[:])
            wv_sb = res.tile([D, D], bf16)
            nc.sync.dma_start(out=wv_sb[:], in_=wvt[:])
            wq_sb = res.tile([D, D], bf16)
            nc.sync.dma_start(out=wq_sb[:], in_=wqt[:])
            wsk_sb = res.tile([D, D], bf16)
            nc.sync.dma_start(out=wsk_sb[:], in_=wskt[:])
            wpr_sb = res.tile([D, D], bf16)
            nc.sync.dma_start(out=wpr_sb[:], in_=wprt[:])
            we2k_sb = res.tile([6, D], bf16)
            nc.sync.dma_start(out=we2k_sb[:], in_=we2k[:])
            we2v_sb = res.tile([6, D], bf16)
            nc.sync.dma_start(out=we2v_sb[:], in_=we2v[:])
            bk_sb = res.tile([D, 1], fp32)
            nc.sync.dma_start(out=bk_sb[:], in_=bkcol[:])
            bv_sb = res.tile([D, 1], fp32)
            nc.sync.dma_start(out=bv_sb[:], in_=bvcol[:])
            qsc0_sb = res.tile([D, 1], fp32)
            nc.sync.dma_start(out=qsc0_sb[:], in_=qsc0[:])
            qsc1_sb = res.tile([D, 1], fp32)
            nc.sync.dma_start(out=qsc1_sb[:], in_=qsc1[:])
            qbc0_sb = res.tile([D, 1], fp32)
            nc.sync.dma_start(out=qbc0_sb[:], in_=qbc0[:])
            qbc1_sb = res.tile([D, 1], fp32)
            nc.sync.dma_start(out=qbc1_sb[:], in_=qbc1[:])
            bsk_sb = res.tile([1, D], bf16)
            nc.sync.dma_start(out=bsk_sb[:], in_=bskrow[:])
            bpr_sb = res.tile([1, D], bf16)
            nc.sync.dma_start(out=bpr_sb[:], in_=bprrow[:])
            wb1_sb = res.tile([P, D], bf16)
            nc.sync.dma_start(out=wb1_sb[:], in_=wb1rep[:])
            wb2_sb = res.tile([P, D], bf16)
            nc.sync.dma_start(out=wb2_sb[:], in_=wb2rep[:])
            iota_sb = res.tile([P, P], bf16)
            nc.sync.dma_start(out=iota_sb[:], in_=iota[:])
            ones_sb = res.tile([1, D], bf16)
            nc.sync.dma_start(out=ones_sb[:], in_=onesrow[:])
            ident_sb = res.tile([P, P], bf16)
            make_identity(nc, ident_sb[:])
            dump_sb = res.tile([P, D], fp32)
            if DUMP:
                nc.vector.memset(dump_sb[:], 0)

            def dmp(name, ap):
                if DUMP == name:
                    w = min(int(ap.shape[-1]), D)
                    nc.vector.tensor_copy(dump_sb[:, :w], ap[..., :w])

            tblv = tbl[:].rearrange("p (n d) -> p n d", d=2)

            for h in range(2):
                # ======== Phase A_h: build K||V table for src half h ========
                if h == 1:
                    # the table rebuild must not overtake pass-0 gathers;
                    # gpsimd reads are not reliably tracked against engine
                    # writes, so drain explicitly
                    tc.strict_bb_all_engine_barrier()
                base = h * HALF
                for g in range(4):
                    w = 8192 if g < 3 else HALF - 3 * 8192
                    xt_t = pa.tile([D, w], bf16, tag="xt_t")
                    nc.sync.dma_start(
                        out=xt_t[:],
                        in_=xt[:, base + g * 8192: base + g * 8192 + w])
                    for s0 in range(0, w, 512):
                        sw = min(512, w - s0)
                        nd0 = g * 8192 + s0
                        kv_ps = pap.tile([P, 1024], fp32, tag="kv_ps")
                        nc.tensor.matmul(out=kv_ps[:, 0:sw],
                                         lhsT=wk_sb[:],
                                         rhs=xt_t[:, s0:s0 + sw],
                                         start=True, stop=True)
                        nc.tensor.matmul(out=kv_ps[:, 512:512 + sw],
                                         lhsT=wv_sb[:],
                                         rhs=xt_t[:, s0:s0 + sw],
                                         start=True, stop=True)
                        nc.vector.tensor_scalar_add(
                            tblv[:, nd0:nd0 + sw, 0], kv_ps[:, 0:sw],
                            bk_sb[:, 0:1])
                        nc.vector.tensor_scalar_add(
                            tblv[:, nd0:nd0 + sw, 1], kv_ps[:, 512:512 + sw],
                            bv_sb[:, 0:1])
                if STAGE < 2:
                    continue

                # ======== Phase B_h: per-block edge work ========
                for b in range(NB):
                    T = meta["T"][h][b]
                    ni = meta["ni"][h][b]
                    to = meta["toff"][h][b] + (sumT[0] if h else 0)
                    qo = meta["qoff"][h][b]

                    # one gather per (pass, block): K and V rows per edge
                    gout = pb.tile([P, T * 256], bf16, tag="gout")
                    nc.gpsimd.ap_gather(
                        out_ap=gout[:, :ni * 2].rearrange(
                            "p (n d) -> p n d", d=2),
                        in_ap=tblv,
                        idxs_ap=idx_sb[h][:, qo:qo + ni // 16],
                        channels=P, num_elems=HALF, d=2, num_idxs=ni)
                    if ni < T * P:
                        nc.vector.memset(gout[:, ni * 2:], 0)

                    ea_blk = pb.tile([6, T * P], bf16, tag="ea_blk")
                    nc.sync.dma_start(out=ea_blk[:],
                                      in_=eat6[:, to * P:(to + T) * P])

                    # qT for this block, pre-scaled by 1/8 and head-masked
                    mm_ps = pbq.tile([P, 384], fp32, tag="mm_ps")
                    q_ps = mm_ps[:, 0:128]
                    nc.tensor.matmul(out=q_ps, lhsT=wq_sb[:],
                                     rhs=xtloc_sb[:, b * P:(b + 1) * P],
                                     start=True, stop=True)
                    qt_sb = pbs.tile([P, 256], bf16, tag="qt_sb")
                    nc.vector.tensor_scalar(
                        out=qt_sb[:, 0:128], in0=q_ps, scalar1=qsc0_sb[:, 0:1],
                        scalar2=qbc0_sb[:, 0:1], op0=AX.mult, op1=AX.add)
                    nc.vector.tensor_scalar(
                        out=qt_sb[:, 128:256], in0=q_ps,
                        scalar1=qsc1_sb[:, 0:1],
                        scalar2=qbc1_sb[:, 0:1], op0=AX.mult, op1=AX.add)

                    if STAGE < 3:
                        continue

                    if h == DH and b == DB:
                        dmp("gk", gout[:].rearrange(
                            "p (n d) -> p n d", d=2)[:, DT * P:(DT + 1) * P, 0])
                        dmp("qt0", qt_sb[:, 0:128])
                    s2b = pb.tile([P, T * P], bf16, tag="s2b")
                    vte = pb.tile([P, T * P], bf16, tag="vte")
                    alpha = pbs.tile([P, 2 * T], fp32, tag="alpha")

                    gv = gout[:].rearrange("p (n d) -> p n d", d=2)
                    t = 0
                    while t < T:
                        tn = min(2, T - t)
                        W = tn * P
                        e2_ps = pbe.tile([P, 512], fp32, tag="e2_ps")
                        nc.tensor.matmul(
                            out=e2_ps[:, 0:W],
                            lhsT=we2k_sb[:],
                            rhs=ea_blk[:, t * P:t * P + W],
                            start=True, stop=True)
                        nc.tensor.matmul(
                            out=e2_ps[:, 256:256 + W],
                            lhsT=we2v_sb[:],
                            rhs=ea_blk[:, t * P:t * P + W],
                            start=True, stop=True)
                        kjt2 = pbs.tile([P, 256], bf16, tag="kjt2")
                        nc.vector.tensor_tensor(
                            out=kjt2[:, :W], in0=gv[:, t * P:t * P + W, 0],
                            in1=e2_ps[:, 0:W], op=AX.add)
                        vjt2 = pbs.tile([P, 256], bf16, tag="vjt2")
                        nc.vector.tensor_tensor(
                            out=vjt2[:, :W], in0=gv[:, t * P:t * P + W, 1],
                            in1=e2_ps[:, 256:256 + W], op=AX.add)

                        # s2 one-hot from dstl for the group
                        nc.vector.tensor_tensor(
                            out=s2b[:, t * P:t * P + W].rearrange(
                                "p (u n) -> p u n", u=tn),
                            in0=dstl_sb[:, to + t:to + t + tn, None]
                            .to_broadcast([P, tn, P]),
                            in1=iota_sb[:, None, :].to_broadcast([P, tn, P]),
                            op=AX.is_equal)

                        # A^T[e, n] per (tile, head); col order t0h0 t0h1 ...
                        a_ps = pbt.tile([P, 512], fp32, tag="a_ps")
                        for tt in range(tn):
                            nc.tensor.matmul(
                                out=a_ps[:, 2 * tt * P:2 * (tt + 1) * P],
                                lhsT=kjt2[:, tt * P:(tt + 1) * P],
                                rhs=qt_sb[:], start=True, stop=True)
                        # alpha[e, t, h] = sum_n A^T * s2
                        asg = pbs.tile([P, 512], fp32, tag="asg")
                        nc.vector.tensor_tensor(
                            out=asg[:, :2 * W].rearrange(
                                "p (u g n) -> p u g n", u=tn, g=2),
                            in0=a_ps[:, :2 * W].rearrange(
                                "p (u g n) -> p u g n", u=tn, g=2),
                            in1=s2b[:, t * P:t * P + W].rearrange(
                                "p (u n) -> p u n", u=tn)[:, :, None, :]
                            .to_broadcast([P, tn, 2, P]),
                            op=AX.mult)
                        nc.vector.tensor_reduce(
                            out=alpha[:, 2 * t:2 * (t + tn)].rearrange(
                                "p (u g) -> p u g", u=tn),
                            in_=asg[:, :2 * W].rearrange(
                                "p (u g n) -> p u g n", u=tn, g=2),
                            axis=mybir.AxisListType.X, op=AX.add)

                        # vj transposed to edge-major
                        vt_ps = pbv.tile([P, 256], bf16, tag="vt_ps")
                        for tt in range(tn):
                            nc.tensor.transpose(
                                out=vt_ps[:, tt * P:(tt + 1) * P],
                                in_=vjt2[:, tt * P:(tt + 1) * P],
                                identity=ident_sb[:])
                        nc.scalar.copy(out=vte[:, t * P:t * P + W],
                                       in_=vt_ps[:, :W])
                        t += tn

                    # softmax numerators for the whole block
                    exb = ex_sb[:, 2 * to:2 * (to + T)]
                    nc.scalar.activation(exb, alpha[:, :2 * T], AF.Exp)

                    if h == DH and b == DB:
                        dmp("alpha", alpha[:, :2 * T])
                        dmp("ex", exb)
                        dmp("vte", vte[:, 0:P])
                    if STAGE < 4:
                        continue

                    # xmat = [vj * ex_h || ex]
                    xmat = pb.tile([P, T * 132], bf16, tag="xmat")
                    xv = xmat[:].rearrange("p (t f) -> p t f", t=T)
                    exg = exb.rearrange("p (t g) -> p t g", t=T)
                    nc.vector.tensor_tensor(
                        out=xv[:, :, 0:128].rearrange(
                            "p t (g c) -> p t g c", g=H),
                        in0=vte[:].rearrange("p (t g c) -> p t g c",
                                             t=T, g=H),
                        in1=exg[:, :, :, None].to_broadcast([P, T, H, C]),
                        op=AX.mult)
                    nc.vector.tensor_copy(xv[:, :, 128:130], exg[:])

                    acc_ps = pba.tile([P, 132], fp32, tag="acc_ps")
                    for t in range(T):
                        nc.tensor.matmul(out=acc_ps[:, 0:130],
                                         lhsT=s2b[:, t * P:(t + 1) * P],
                                         rhs=xmat[:, t * 132:t * 132 + 130],
                                         start=(t == 0), stop=(t == T - 1))
                    ab = acc_sb[:, b * 132:b * 132 + 132]
                    if h == 0:
                        nc.vector.tensor_copy(ab[:, 0:130], acc_ps[:, 0:130])
                    else:
                        nc.vector.tensor_tensor(out=ab[:, 0:130],
                                                in0=ab[:, 0:130],
                                                in1=acc_ps[:, 0:130],
                                                op=AX.add)
                    if h == DH and b == DB:
                        dmp("acc", ab[:, 0:128])
                        dmp("accd", ab[:, 112:130])

                    if h == 0 or STAGE < 5:
                        continue

                    # ======== epilogue for block b ========
                    den = pbs.tile([P, 2], fp32, tag="den")
                    nc.vector.tensor_scalar_add(den[:], ab[:, 128:130], 1e-30)
                    denr = pbs.tile([P, 2], fp32, tag="denr")
                    nc.vector.reciprocal(denr[:], den[:])
                    oa = pbs.tile([P, D], bf16, tag="oa")
                    for hh in range(H):
                        nc.vector.tensor_scalar_mul(
                            oa[:, hh * C:(hh + 1) * C],
                            ab[:, hh * C:(hh + 1) * C],
                            denr[:, hh:hh + 1])

                    xr_ps = mm_ps[:, 128:256]
                    nc.tensor.matmul(out=xr_ps,
                                     lhsT=xtloc_sb[:, b * P:(b + 1) * P],
                                     rhs=wsk_sb[:], start=True, stop=False)
                    nc.tensor.matmul(out=xr_ps, lhsT=ones_sb[:],
                                     rhs=bsk_sb[:], start=False, stop=True)
                    xr_sb = pbs.tile([P, D], bf16, tag="xr_sb")
                    nc.scalar.copy(out=xr_sb[:], in_=xr_ps)

                    bp = pbs.tile([P, 2], fp32, tag="bp")
                    sc2 = pbs.tile([P, D], bf16, tag="sc2")
                    nc.vector.scalar_tensor_tensor(
                        out=sc2[:], in0=oa[:], scalar=1.0, in1=wb1_sb[:],
                        op0=AX.bypass, op1=AX.mult, accum_out=bp[:, 0:1])
                    sc3 = pbs.tile([P, D], bf16, tag="sc3")
                    nc.vector.scalar_tensor_tensor(
                        out=sc3[:], in0=xr_sb[:], scalar=-1.0, in1=wb2_sb[:],
                        op0=AX.mult, op1=AX.mult, accum_out=bp[:, 1:2])
                    ebt = pbs.tile([P, 1], fp32, tag="ebt")
                    nc.scalar.activation(ebt[:], bp[:, 0:1], AF.Exp,
                                         bias=bp[:, 1:2], scale=-1.0)
                    ebt1 = pbs.tile([P, 1], fp32, tag="ebt1")
                    nc.vector.tensor_scalar_add(ebt1[:], ebt[:], 1.0)
                    beta = pbs.tile([P, 1], fp32, tag="beta")
                    nc.vector.reciprocal(beta[:], ebt1[:])

                    diff = pbs.tile([P, D], bf16, tag="diff")
                    nc.vector.tensor_tensor(out=diff[:], in0=xr_sb[:],
                                            in1=oa[:], op=AX.subtract)
                    y_sb = pbs.tile([P, D], bf16, tag="y_sb")
                    nc.vector.scalar_tensor_tensor(
                        out=y_sb[:], in0=diff[:], scalar=beta[:, 0:1],
                        in1=oa[:], op0=AX.mult, op1=AX.add)
                    if b == DB:
                        dmp("oa", oa[:])
                        dmp("xr", xr_sb[:])
                        dmp("bp", bp[:])
                        dmp("beta", beta[:])
                        dmp("ydump", y_sb[:])

                    yt_ps = pbv.tile([P, D], bf16, tag="vt_ps")
                    nc.tensor.transpose(out=yt_ps[:], in_=y_sb[:],
                                        identity=ident_sb[:])
                    yt_sb = pbs.tile([P, D], bf16, tag="yt_sb")
                    nc.scalar.copy(out=yt_sb[:], in_=yt_ps[:])
                    yp_ps = mm_ps[:, 256:384]
                    nc.tensor.matmul(out=yp_ps, lhsT=yt_sb[:],
                                     rhs=wpr_sb[:], start=True, stop=False)
                    nc.tensor.matmul(out=yp_ps, lhsT=ones_sb[:],
                                     rhs=bpr_sb[:], start=False, stop=True)
                    o_sb = pbs.tile([P, D], fp32, tag="o_sb")
                    if b % 2 == 0:
                        nc.scalar.copy(out=o_sb[:], in_=yp_ps)
                    else:
                        nc.vector.tensor_copy(o_sb[:], yp_ps)
                    if b == DB:
                        dmp("osb", o_sb[:])
                    nc.sync.dma_start(out=out[b * P:(b + 1) * P, :],
                                      in_=o_sb[:])
            if DUMP:
                nc.sync.dma_start(out=out[0:P, :], in_=dump_sb[:])

    nc.compile()
    return nc


_CACHE = {}


def kernel(**inputs):
    from concourse.bass_utils import run_bass_kernel_spmd

    per_core, meta = _prep_host(**inputs)
    key = (tuple(meta["ni"][0]), tuple(meta["ni"][1]))
    if key not in _CACHE:
        _CACHE[key] = _build_program(meta)
    nc = _CACHE[key]
    res = run_bass_kernel_spmd(nc, per_core, core_ids=list(range(NCORES)))
    full = np.concatenate([res.results[c]["out"] for c in range(NCORES)], axis=0)
    return np.ascontiguousarray(full[:N]).astype(np.float32)
